# revision 6
# baseline (speedup 1.0000x reference)
"""Contrastive-loss kernel for Trainium2 (8 NeuronCores, SPMD, raw Bass).

loss = sum_{i != j} dist[i,j] / (2 N (N-1)) with
dist[i,j] = ||x_i||^2 + ||y_j||^2 - 2 x_i . y_j.

The full off-diagonal sum collapses algebraically:
    sum_{i,j} dist = N*(Sx + Sy) - 2 * sx . sy
    diag          = Sx + Sy - 2 * tr
with Sx = sum_i ||x_i||^2, sx = sum_i x_i (column sums), tr = sum_i x_i.y_i.
So the device only performs O(N*D) reductions over feature1/feature2 —
each core reads its 1/8 row-shard of both tensors (1 MiB) and returns
tiny partials; the host combines them in float64.

Per-core device program (shard = [1024, 128] of each tensor), raw Bass
(explicit semaphores; this toolchain accepts at most one sync-wait per
instruction, which Tile's fused waits / tail drain violate):
  - SP:  DMA both shards into SBUF as [128 part, 1024 free] (contiguous),
         then DMA the partial results out when producers signal.
  - ACT: Square activation with accum_out -> row-sums of x^2 and y^2.
  - DVE: x*y multiply + reduce          -> row-sums of x*y.
  - PE:  ones^T @ X / Y matmuls         -> per-(group,col) partition sums
         (512-wide, one PSUM bank each); DVE/ACT copy them to SBUF.
"""

import numpy as np

N, D = 8192, 128
NCORES = 8
ROWS = N // NCORES          # 1024 rows per core per tensor
P = 128                     # SBUF partitions
KG = ROWS // P              # 8 row-groups folded into the free dim
FREE = KG * D               # 1024 free elements per partition
HALF = FREE // 2            # 512 = one PSUM bank of f32

_NC_CACHE = {}


def _build_bass():
    from contextlib import ExitStack

    import concourse.bass as bass
    from concourse import mybir

    f32 = mybir.dt.float32
    nc = bass.Bass()
    x = nc.dram_tensor("x", [ROWS, D], f32, kind="ExternalInput")
    y = nc.dram_tensor("y", [ROWS, D], f32, kind="ExternalInput")
    out_cols = nc.dram_tensor("out_cols", [1, 2 * FREE], f32,
                              kind="ExternalOutput")
    out_rsa = nc.dram_tensor("out_rsa", [P, 2], f32, kind="ExternalOutput")
    out_rsv = nc.dram_tensor("out_rsv", [P, 1], f32, kind="ExternalOutput")

    xr = x.rearrange("(p k) d -> p (k d)", p=P)
    yr = y.rearrange("(p k) d -> p (k d)", p=P)

    ones = nc.const_aps.tensor(1.0, (P, 1), f32)

    with ExitStack() as ctx:
        X = ctx.enter_context(nc.sbuf_tensor("X", [P, FREE], f32))
        Y = ctx.enter_context(nc.sbuf_tensor("Y", [P, FREE], f32))
        scr_act = ctx.enter_context(nc.sbuf_tensor("scr_act", [P, FREE], f32))
        scr_dve = ctx.enter_context(nc.sbuf_tensor("scr_dve", [P, FREE], f32))
        rs_a = ctx.enter_context(nc.sbuf_tensor("rs_a", [P, 2], f32))
        rs_v = ctx.enter_context(nc.sbuf_tensor("rs_v", [P, 1], f32))
        outsb = ctx.enter_context(nc.sbuf_tensor("outsb", [1, 2 * FREE], f32))
        px0 = ctx.enter_context(nc.psum_tensor([1, HALF], f32))
        px1 = ctx.enter_context(nc.psum_tensor([1, HALF], f32))
        py0 = ctx.enter_context(nc.psum_tensor([1, HALF], f32))
        py1 = ctx.enter_context(nc.psum_tensor([1, HALF], f32))

        din = ctx.enter_context(nc.semaphore("din"))
        dout = ctx.enter_context(nc.semaphore("dout"))
        pe_sem = ctx.enter_context(nc.semaphore("pe_sem"))
        a_sem = ctx.enter_context(nc.semaphore("a_sem"))
        v_sem = ctx.enter_context(nc.semaphore("v_sem"))
        copy_sem = ctx.enter_context(nc.semaphore("copy_sem"))

        with nc.Block() as block:

            @block.sync
            def _(sync):
                sync.dma_start(out=X[:], in_=xr).then_inc(din, 16)
                sync.dma_start(out=Y[:], in_=yr).then_inc(din, 16)
                # Partial results out, each gated on its producer.
                sync.wait_ge(copy_sem, 4)
                sync.dma_start(out=out_cols[:, :], in_=outsb[:]).then_inc(
                    dout, 16)
                sync.wait_ge(a_sem, 1)
                sync.dma_start(out=out_rsa[:, :], in_=rs_a[:]).then_inc(
                    dout, 16)
                sync.wait_ge(v_sem, 1)
                sync.dma_start(out=out_rsv[:, :], in_=rs_v[:]).then_inc(
                    dout, 16)
                sync.wait_ge(dout, 48)

            @block.tensor
            def _(tensor):
                tensor.wait_ge(din, 32)
                nc.tensor.matmul(px0[:], ones, X[:, 0:HALF],
                                 start=True, stop=True).then_inc(pe_sem, 1)
                nc.tensor.matmul(px1[:], ones, X[:, HALF:FREE],
                                 start=True, stop=True).then_inc(pe_sem, 1)
                nc.tensor.matmul(py0[:], ones, Y[:, 0:HALF],
                                 start=True, stop=True).then_inc(pe_sem, 1)
                nc.tensor.matmul(py1[:], ones, Y[:, HALF:FREE],
                                 start=True, stop=True).then_inc(pe_sem, 1)

            @block.scalar
            def _(scalar):
                scalar.wait_ge(din, 32)
                nc.scalar.activation(
                    out=scr_act[:], in_=X[:],
                    func=mybir.ActivationFunctionType.Square,
                    accum_out=rs_a[:, 0:1])
                nc.scalar.activation(
                    out=scr_act[:], in_=Y[:],
                    func=mybir.ActivationFunctionType.Square,
                    accum_out=rs_a[:, 1:2]).then_inc(a_sem, 1)
                scalar.wait_ge(pe_sem, 3)
                nc.scalar.copy(out=outsb[0:1, 2 * HALF:3 * HALF],
                               in_=py0[:]).then_inc(copy_sem, 1)
                scalar.wait_ge(pe_sem, 4)
                nc.scalar.copy(out=outsb[0:1, 3 * HALF:4 * HALF],
                               in_=py1[:]).then_inc(copy_sem, 1)

            @block.vector
            def _(vector):
                vector.wait_ge(din, 32)
                nc.vector.tensor_mul(out=scr_dve[:], in0=X[:], in1=Y[:])
                nc.vector.reduce_sum(rs_v[:, 0:1], scr_dve[:],
                                     axis=mybir.AxisListType.X).then_inc(
                    v_sem, 1)
                vector.wait_ge(pe_sem, 1)
                nc.vector.tensor_copy(out=outsb[0:1, 0:HALF],
                                      in_=px0[:]).then_inc(copy_sem, 1)
                vector.wait_ge(pe_sem, 2)
                nc.vector.tensor_copy(out=outsb[0:1, HALF:2 * HALF],
                                      in_=px1[:]).then_inc(copy_sem, 1)

    return nc


def _get_nc():
    if "nc" not in _NC_CACHE:
        _NC_CACHE["nc"] = _build_bass()
    return _NC_CACHE["nc"]


def _run_device(f1, f2, **spmd_kwargs):
    from concourse.bass_utils import run_bass_kernel_spmd

    nc = _get_nc()
    in_maps = [
        {"x": f1[c * ROWS:(c + 1) * ROWS], "y": f2[c * ROWS:(c + 1) * ROWS]}
        for c in range(NCORES)
    ]
    return run_bass_kernel_spmd(nc, in_maps, core_ids=list(range(NCORES)),
                                **spmd_kwargs)


def _combine(results):
    sx = np.zeros(D, np.float64)
    sy = np.zeros(D, np.float64)
    Sx = Sy = tr = 0.0
    for r in results:
        cols = r["out_cols"][0].astype(np.float64)
        sx += cols[0:FREE].reshape(KG, D).sum(axis=0)
        sy += cols[FREE:2 * FREE].reshape(KG, D).sum(axis=0)
        rsa = r["out_rsa"].astype(np.float64)
        Sx += rsa[:, 0].sum()
        Sy += rsa[:, 1].sum()
        tr += r["out_rsv"].astype(np.float64).sum()
    total = N * (Sx + Sy) - 2.0 * float(sx @ sy) - (Sx + Sy - 2.0 * tr)
    loss = total / 2.0 / (N * (N - 1))
    return np.asarray(loss, dtype=np.float32)


def kernel(feature1, feature2, label=None, **_unused):
    f1 = np.ascontiguousarray(np.asarray(feature1, dtype=np.float32))
    f2 = np.ascontiguousarray(np.asarray(feature2, dtype=np.float32))
    res = _run_device(f1, f2)
    return _combine(res.results)


# revision 7
# speedup vs baseline: 1.6437x; 1.6437x over previous
"""Contrastive-loss kernel for Trainium2 (8 NeuronCores, SPMD, raw Bass).

loss = sum_{i != j} dist[i,j] / (2 N (N-1)) with
dist[i,j] = ||x_i||^2 + ||y_j||^2 - 2 x_i . y_j.

The full off-diagonal sum collapses algebraically:
    sum_{i,j} dist = N*(Sx + Sy) - 2 * sx . sy
    diag          = Sx + Sy - 2 * tr
with Sx = sum_i ||x_i||^2, sx = sum_i x_i (column sums), tr = sum_i x_i.y_i.
So the device only performs O(N*D) reductions over feature1/feature2 —
each core reads its 1/8 row-shard of both tensors (1 MiB) and returns
tiny partials; the host combines them in float64.

Per-core device program (shard = [1024, 128] of each tensor), raw Bass
(explicit semaphores; this toolchain accepts at most one sync-wait per
instruction, which Tile's fused waits / tail drain violate):
  - SP:  DMA the x shard in; DMA the single packed [1, 2051] result out.
  - ACT: issues the y-shard DMA on its own HWDGE ring, prewarms the
         Square PWP table while DMAs fly, then Square+accum row-sums
         of x^2 / y^2, and copies two PSUM results to SBUF.
  - DVE: x*y multiply + reduce -> row-sums of x*y; PSUM->SBUF copies.
  - PE:  ones^T @ X / Y matmuls -> per-(group,col) partition sums
         (512-wide, one PSUM bank each) plus two tiny matmuls that
         collapse the [128,2]/[128,1] row-sum tiles to scalars so the
         output DMA is one contiguous descriptor.
"""

import numpy as np

N, D = 8192, 128
NCORES = 8
ROWS = N // NCORES          # 1024 rows per core per tensor
P = 128                     # SBUF partitions
KG = ROWS // P              # 8 row-groups folded into the free dim
FREE = KG * D               # 1024 free elements per partition
HALF = FREE // 2            # 512 = one PSUM bank of f32
OUT_LEN = 2 * FREE + 3      # [cols_x | cols_y | Sx, Sy, tr]

_NC_CACHE = {}


def _build_bass():
    from contextlib import ExitStack

    import concourse.bass as bass
    from concourse import mybir

    f32 = mybir.dt.float32
    SQ = mybir.ActivationFunctionType.Square
    nc = bass.Bass()
    x = nc.dram_tensor("x", [ROWS, D], f32, kind="ExternalInput")
    y = nc.dram_tensor("y", [ROWS, D], f32, kind="ExternalInput")
    out = nc.dram_tensor("out", [1, OUT_LEN], f32, kind="ExternalOutput")

    xr = x.rearrange("(p k) d -> p (k d)", p=P)
    yr = y.rearrange("(p k) d -> p (k d)", p=P)

    ones = nc.const_aps.tensor(1.0, (P, 1), f32)

    with ExitStack() as ctx:
        X = ctx.enter_context(nc.sbuf_tensor("X", [P, FREE], f32))
        Y = ctx.enter_context(nc.sbuf_tensor("Y", [P, FREE], f32))
        scr_act = ctx.enter_context(nc.sbuf_tensor("scr_act", [P, FREE], f32))
        scr_dve = ctx.enter_context(nc.sbuf_tensor("scr_dve", [P, FREE], f32))
        warm = ctx.enter_context(nc.sbuf_tensor("warm", [P, 1], f32))
        rs_a = ctx.enter_context(nc.sbuf_tensor("rs_a", [P, 2], f32))
        rs_v = ctx.enter_context(nc.sbuf_tensor("rs_v", [P, 1], f32))
        outsb = ctx.enter_context(nc.sbuf_tensor("outsb", [1, OUT_LEN], f32))
        px0 = ctx.enter_context(nc.psum_tensor([1, HALF], f32))
        px1 = ctx.enter_context(nc.psum_tensor([1, HALF], f32))
        py0 = ctx.enter_context(nc.psum_tensor([1, HALF], f32))
        py1 = ctx.enter_context(nc.psum_tensor([1, HALF], f32))
        prs_a = ctx.enter_context(nc.psum_tensor([1, 2], f32))
        prs_b = ctx.enter_context(nc.psum_tensor([1, 1], f32))

        dinx = ctx.enter_context(nc.semaphore("dinx"))
        diny = ctx.enter_context(nc.semaphore("diny"))
        dout = ctx.enter_context(nc.semaphore("dout"))
        pe_sem = ctx.enter_context(nc.semaphore("pe_sem"))
        a_sem = ctx.enter_context(nc.semaphore("a_sem"))
        v_sem = ctx.enter_context(nc.semaphore("v_sem"))
        copy_sem = ctx.enter_context(nc.semaphore("copy_sem"))

        with nc.Block() as block:

            @block.sync
            def _(sync):
                sync.dma_start(out=X[:], in_=xr).then_inc(dinx, 16)
                sync.wait_ge(copy_sem, 6)
                sync.dma_start(out=out[:, :], in_=outsb[:]).then_inc(dout, 16)
                sync.wait_ge(dout, 16)

            @block.scalar
            def _(scalar):
                # y load on the ACT HWDGE ring, in parallel with x on SP's.
                scalar.dma_start(out=Y[:], in_=yr).then_inc(diny, 16)
                # Prewarm the Square PWP table while the DMAs fly.
                nc.scalar.activation(out=warm[:], in_=warm[:], func=SQ)
                scalar.wait_ge(dinx, 16)
                nc.scalar.activation(out=scr_act[:], in_=X[:], func=SQ,
                                     accum_out=rs_a[:, 0:1])
                scalar.wait_ge(diny, 16)
                nc.scalar.activation(out=scr_act[:], in_=Y[:], func=SQ,
                                     accum_out=rs_a[:, 1:2]).then_inc(a_sem, 1)
                scalar.wait_ge(pe_sem, 3)
                nc.scalar.copy(out=outsb[0:1, 2 * HALF:3 * HALF],
                               in_=py0[:]).then_inc(copy_sem, 1)
                scalar.wait_ge(pe_sem, 4)
                nc.scalar.copy(out=outsb[0:1, 3 * HALF:4 * HALF],
                               in_=py1[:]).then_inc(copy_sem, 1)

            @block.tensor
            def _(tensor):
                tensor.wait_ge(dinx, 16)
                nc.tensor.matmul(px0[:], ones, X[:, 0:HALF],
                                 start=True, stop=True).then_inc(pe_sem, 1)
                nc.tensor.matmul(px1[:], ones, X[:, HALF:FREE],
                                 start=True, stop=True).then_inc(pe_sem, 1)
                tensor.wait_ge(diny, 16)
                nc.tensor.matmul(py0[:], ones, Y[:, 0:HALF],
                                 start=True, stop=True).then_inc(pe_sem, 1)
                nc.tensor.matmul(py1[:], ones, Y[:, HALF:FREE],
                                 start=True, stop=True).then_inc(pe_sem, 1)
                tensor.wait_ge(a_sem, 1)
                nc.tensor.matmul(prs_a[:], ones, rs_a[:],
                                 start=True, stop=True).then_inc(pe_sem, 1)
                tensor.wait_ge(v_sem, 1)
                nc.tensor.matmul(prs_b[:], ones, rs_v[:],
                                 start=True, stop=True).then_inc(pe_sem, 1)

            @block.vector
            def _(vector):
                vector.wait_ge(dinx, 16)
                vector.wait_ge(diny, 16)
                nc.vector.tensor_mul(out=scr_dve[:], in0=X[:], in1=Y[:])
                nc.vector.reduce_sum(rs_v[:, 0:1], scr_dve[:],
                                     axis=mybir.AxisListType.X).then_inc(
                    v_sem, 1)
                vector.wait_ge(pe_sem, 1)
                nc.vector.tensor_copy(out=outsb[0:1, 0:HALF],
                                      in_=px0[:]).then_inc(copy_sem, 1)
                vector.wait_ge(pe_sem, 2)
                nc.vector.tensor_copy(out=outsb[0:1, HALF:2 * HALF],
                                      in_=px1[:]).then_inc(copy_sem, 1)
                vector.wait_ge(pe_sem, 6)
                nc.vector.tensor_copy(out=outsb[0:1, 2 * FREE:2 * FREE + 2],
                                      in_=prs_a[:]).then_inc(copy_sem, 1)
                nc.vector.tensor_copy(out=outsb[0:1, 2 * FREE + 2:OUT_LEN],
                                      in_=prs_b[:]).then_inc(copy_sem, 1)

    return nc


def _get_nc():
    if "nc" not in _NC_CACHE:
        _NC_CACHE["nc"] = _build_bass()
    return _NC_CACHE["nc"]


def _run_device(f1, f2, **spmd_kwargs):
    from concourse.bass_utils import run_bass_kernel_spmd

    nc = _get_nc()
    in_maps = [
        {"x": f1[c * ROWS:(c + 1) * ROWS], "y": f2[c * ROWS:(c + 1) * ROWS]}
        for c in range(NCORES)
    ]
    return run_bass_kernel_spmd(nc, in_maps, core_ids=list(range(NCORES)),
                                **spmd_kwargs)


def _combine(results):
    sx = np.zeros(D, np.float64)
    sy = np.zeros(D, np.float64)
    Sx = Sy = tr = 0.0
    for r in results:
        o = r["out"][0].astype(np.float64)
        sx += o[0:FREE].reshape(KG, D).sum(axis=0)
        sy += o[FREE:2 * FREE].reshape(KG, D).sum(axis=0)
        Sx += o[2 * FREE]
        Sy += o[2 * FREE + 1]
        tr += o[2 * FREE + 2]
    total = N * (Sx + Sy) - 2.0 * float(sx @ sy) - (Sx + Sy - 2.0 * tr)
    loss = total / 2.0 / (N * (N - 1))
    return np.asarray(loss, dtype=np.float32)


def kernel(feature1, feature2, label=None, **_unused):
    f1 = np.ascontiguousarray(np.asarray(feature1, dtype=np.float32))
    f2 = np.ascontiguousarray(np.asarray(feature2, dtype=np.float32))
    res = _run_device(f1, f2)
    return _combine(res.results)


# revision 8
# speedup vs baseline: 1.6707x; 1.0164x over previous
"""Contrastive-loss kernel for Trainium2 (8 NeuronCores, SPMD, raw Bass).

loss = sum_{i != j} dist[i,j] / (2 N (N-1)) with
dist[i,j] = ||x_i||^2 + ||y_j||^2 - 2 x_i . y_j.

The full off-diagonal sum collapses algebraically:
    sum_{i,j} dist = N*(Sx + Sy) - 2 * sx . sy
    diag          = Sx + Sy - 2 * tr
with Sx = sum_i ||x_i||^2, sx = sum_i x_i (column sums), tr = sum_i x_i.y_i.
So the device only performs O(N*D) reductions over feature1/feature2 —
each core reads its 1/8 row-shard of both tensors (1 MiB) and returns
tiny partials; the host combines them in float64.

Per-core device program (shard = [1024, 128] of each tensor), raw Bass
(explicit semaphores; this toolchain accepts at most one sync-wait per
instruction, which Tile's fused waits / tail drain violate). Inputs are
loaded in two 256 KiB chunks per tensor (x on the SP HWDGE ring, y on
the ACT ring, FIFO within each ring) so compute starts after the first
chunk lands:
  - ACT: prewarms the Square PWP table during the DMAs, then per-chunk
         Square+accum row-sums of x^2 / y^2, and two PSUM->SBUF copies.
  - DVE: per-chunk x*y multiply + reduce row-sums; PSUM->SBUF copies.
  - PE:  ones^T @ chunk matmuls -> per-(group,col) partition sums (one
         PSUM bank each) plus two tiny matmuls collapsing the [128,4]/
         [128,2] row-sum tiles to scalars so the result leaves in one
         contiguous [1, 2054] DMA descriptor.
"""

import numpy as np

N, D = 8192, 128
NCORES = 8
ROWS = N // NCORES          # 1024 rows per core per tensor
P = 128                     # SBUF partitions
KG = ROWS // P              # 8 row-groups folded into the free dim
FREE = KG * D               # 1024 free elements per partition
HALF = FREE // 2            # 512 = one PSUM bank of f32
OUT_LEN = 2 * FREE + 6      # [cols_x | cols_y | Sx0,Sx1,Sy0,Sy1 | tr0,tr1]

_NC_CACHE = {}


def _build_bass():
    from contextlib import ExitStack

    import concourse.bass as bass
    from concourse import mybir

    f32 = mybir.dt.float32
    SQ = mybir.ActivationFunctionType.Square
    nc = bass.Bass()
    x = nc.dram_tensor("x", [ROWS, D], f32, kind="ExternalInput")
    y = nc.dram_tensor("y", [ROWS, D], f32, kind="ExternalInput")
    out = nc.dram_tensor("out", [1, OUT_LEN], f32, kind="ExternalOutput")

    xr = x.rearrange("(p k) d -> p (k d)", p=P)
    yr = y.rearrange("(p k) d -> p (k d)", p=P)

    ones = nc.const_aps.tensor(1.0, (P, 1), f32)

    with ExitStack() as ctx:
        X = ctx.enter_context(nc.sbuf_tensor("X", [P, FREE], f32))
        Y = ctx.enter_context(nc.sbuf_tensor("Y", [P, FREE], f32))
        scr_act = ctx.enter_context(nc.sbuf_tensor("scr_act", [P, HALF], f32))
        scr_dve = ctx.enter_context(nc.sbuf_tensor("scr_dve", [P, HALF], f32))
        warm = ctx.enter_context(nc.sbuf_tensor("warm", [P, 1], f32))
        rs_a = ctx.enter_context(nc.sbuf_tensor("rs_a", [P, 4], f32))
        rs_v = ctx.enter_context(nc.sbuf_tensor("rs_v", [P, 2], f32))
        outsb = ctx.enter_context(nc.sbuf_tensor("outsb", [1, OUT_LEN], f32))
        px0 = ctx.enter_context(nc.psum_tensor([1, HALF], f32))
        px1 = ctx.enter_context(nc.psum_tensor([1, HALF], f32))
        py0 = ctx.enter_context(nc.psum_tensor([1, HALF], f32))
        py1 = ctx.enter_context(nc.psum_tensor([1, HALF], f32))
        prs_a = ctx.enter_context(nc.psum_tensor([1, 4], f32))
        prs_b = ctx.enter_context(nc.psum_tensor([1, 2], f32))

        dx0 = ctx.enter_context(nc.semaphore("dx0"))
        dx1 = ctx.enter_context(nc.semaphore("dx1"))
        dy0 = ctx.enter_context(nc.semaphore("dy0"))
        dy1 = ctx.enter_context(nc.semaphore("dy1"))
        dout = ctx.enter_context(nc.semaphore("dout"))
        pe_sem = ctx.enter_context(nc.semaphore("pe_sem"))
        a_sem = ctx.enter_context(nc.semaphore("a_sem"))
        v_sem = ctx.enter_context(nc.semaphore("v_sem"))
        copy_sem = ctx.enter_context(nc.semaphore("copy_sem"))

        with nc.Block() as block:

            @block.sync
            def _(sync):
                sync.dma_start(out=X[:, 0:HALF],
                               in_=xr[:, 0:HALF]).then_inc(dx0, 16)
                sync.dma_start(out=X[:, HALF:FREE],
                               in_=xr[:, HALF:FREE]).then_inc(dx1, 16)
                sync.wait_ge(copy_sem, 6)
                sync.dma_start(out=out[:, :], in_=outsb[:]).then_inc(dout, 16)
                sync.wait_ge(dout, 16)

            @block.scalar
            def _(scalar):
                # y chunks on the ACT HWDGE ring, parallel to x on SP's.
                scalar.dma_start(out=Y[:, 0:HALF],
                                 in_=yr[:, 0:HALF]).then_inc(dy0, 16)
                scalar.dma_start(out=Y[:, HALF:FREE],
                                 in_=yr[:, HALF:FREE]).then_inc(dy1, 16)
                # Prewarm the Square PWP table while the DMAs fly.
                nc.scalar.activation(out=warm[:], in_=warm[:], func=SQ)
                scalar.wait_ge(dx0, 16)
                nc.scalar.activation(out=scr_act[:], in_=X[:, 0:HALF],
                                     func=SQ, accum_out=rs_a[:, 0:1])
                scalar.wait_ge(dx1, 16)
                nc.scalar.activation(out=scr_act[:], in_=X[:, HALF:FREE],
                                     func=SQ, accum_out=rs_a[:, 1:2])
                scalar.wait_ge(dy0, 16)
                nc.scalar.activation(out=scr_act[:], in_=Y[:, 0:HALF],
                                     func=SQ, accum_out=rs_a[:, 2:3])
                scalar.wait_ge(dy1, 16)
                nc.scalar.activation(out=scr_act[:], in_=Y[:, HALF:FREE],
                                     func=SQ,
                                     accum_out=rs_a[:, 3:4]).then_inc(a_sem, 1)
                scalar.wait_ge(pe_sem, 3)
                nc.scalar.copy(out=outsb[0:1, 2 * HALF:3 * HALF],
                               in_=py0[:]).then_inc(copy_sem, 1)
                scalar.wait_ge(pe_sem, 4)
                nc.scalar.copy(out=outsb[0:1, 3 * HALF:4 * HALF],
                               in_=py1[:]).then_inc(copy_sem, 1)

            @block.tensor
            def _(tensor):
                tensor.wait_ge(dx0, 16)
                nc.tensor.matmul(px0[:], ones, X[:, 0:HALF],
                                 start=True, stop=True).then_inc(pe_sem, 1)
                tensor.wait_ge(dx1, 16)
                nc.tensor.matmul(px1[:], ones, X[:, HALF:FREE],
                                 start=True, stop=True).then_inc(pe_sem, 1)
                tensor.wait_ge(dy0, 16)
                nc.tensor.matmul(py0[:], ones, Y[:, 0:HALF],
                                 start=True, stop=True).then_inc(pe_sem, 1)
                tensor.wait_ge(dy1, 16)
                nc.tensor.matmul(py1[:], ones, Y[:, HALF:FREE],
                                 start=True, stop=True).then_inc(pe_sem, 1)
                tensor.wait_ge(a_sem, 1)
                nc.tensor.matmul(prs_a[:], ones, rs_a[:],
                                 start=True, stop=True).then_inc(pe_sem, 1)
                tensor.wait_ge(v_sem, 1)
                nc.tensor.matmul(prs_b[:], ones, rs_v[:],
                                 start=True, stop=True).then_inc(pe_sem, 1)

            @block.vector
            def _(vector):
                vector.wait_ge(dx0, 16)
                vector.wait_ge(dy0, 16)
                nc.vector.tensor_mul(out=scr_dve[:], in0=X[:, 0:HALF],
                                     in1=Y[:, 0:HALF])
                nc.vector.reduce_sum(rs_v[:, 0:1], scr_dve[:],
                                     axis=mybir.AxisListType.X)
                vector.wait_ge(pe_sem, 1)
                nc.vector.tensor_copy(out=outsb[0:1, 0:HALF],
                                      in_=px0[:]).then_inc(copy_sem, 1)
                vector.wait_ge(dx1, 16)
                vector.wait_ge(dy1, 16)
                nc.vector.tensor_mul(out=scr_dve[:], in0=X[:, HALF:FREE],
                                     in1=Y[:, HALF:FREE])
                nc.vector.reduce_sum(rs_v[:, 1:2], scr_dve[:],
                                     axis=mybir.AxisListType.X).then_inc(
                    v_sem, 1)
                vector.wait_ge(pe_sem, 2)
                nc.vector.tensor_copy(out=outsb[0:1, HALF:2 * HALF],
                                      in_=px1[:]).then_inc(copy_sem, 1)
                vector.wait_ge(pe_sem, 6)
                nc.vector.tensor_copy(out=outsb[0:1, 2 * FREE:2 * FREE + 4],
                                      in_=prs_a[:]).then_inc(copy_sem, 1)
                nc.vector.tensor_copy(out=outsb[0:1, 2 * FREE + 4:OUT_LEN],
                                      in_=prs_b[:]).then_inc(copy_sem, 1)

    return nc


def _get_nc():
    if "nc" not in _NC_CACHE:
        _NC_CACHE["nc"] = _build_bass()
    return _NC_CACHE["nc"]


def _run_device(f1, f2, **spmd_kwargs):
    from concourse.bass_utils import run_bass_kernel_spmd

    nc = _get_nc()
    in_maps = [
        {"x": f1[c * ROWS:(c + 1) * ROWS], "y": f2[c * ROWS:(c + 1) * ROWS]}
        for c in range(NCORES)
    ]
    return run_bass_kernel_spmd(nc, in_maps, core_ids=list(range(NCORES)),
                                **spmd_kwargs)


def _combine(results):
    sx = np.zeros(D, np.float64)
    sy = np.zeros(D, np.float64)
    Sx = Sy = tr = 0.0
    for r in results:
        o = r["out"][0].astype(np.float64)
        sx += o[0:FREE].reshape(KG, D).sum(axis=0)
        sy += o[FREE:2 * FREE].reshape(KG, D).sum(axis=0)
        Sx += o[2 * FREE] + o[2 * FREE + 1]
        Sy += o[2 * FREE + 2] + o[2 * FREE + 3]
        tr += o[2 * FREE + 4] + o[2 * FREE + 5]
    total = N * (Sx + Sy) - 2.0 * float(sx @ sy) - (Sx + Sy - 2.0 * tr)
    loss = total / 2.0 / (N * (N - 1))
    return np.asarray(loss, dtype=np.float32)


def kernel(feature1, feature2, label=None, **_unused):
    f1 = np.ascontiguousarray(np.asarray(feature1, dtype=np.float32))
    f2 = np.ascontiguousarray(np.asarray(feature2, dtype=np.float32))
    res = _run_device(f1, f2)
    return _combine(res.results)
